# revision 2
# baseline (speedup 1.0000x reference)
"""Multi-head self-attention (B=2, S=2048, D=1024, H=16, causal) on 8 TRN2 cores.

Sharding: core c handles batch b=c//4 and head-group g=c%4 (4 heads each).
Host pre-transposes x and the weight slices so the kernel never needs an
on-chip transpose, and pre-converts them to bf16 (PE moving/stationary
streams run at 1 cycle/row in bf16 vs ~1.4-2.1 for fp32r; DMA halves):
  xT   [1024, 2048] = x[b].T
  wqT/wkT/wvT [1024, 256] = W.T[:, g*256:(g+1)*256]
  woT  [256, 1024] = Wo[:, g*256:(g+1)*256].T
Host sums the 4 per-group partial outputs per batch at the end.

On-chip dataflow per core:
  qT/kT [256, 2048] (head dim on partitions), v [2048, 4*65] (with a ones
  column appended per head so the PV matmul also accumulates the softmax
  denominator in psum row 64).  Scores are computed transposed
  (scoresT[j, i]) so softmax needs no transpose at all; there is no
  max-subtraction (scores are O(+-6), exp is safe in fp32).
  The softmax 1/denominator is broadcast across the 64 head-dim partitions
  on the (otherwise idle) GpSimd engine, keeping the PE free of the K=1
  broadcast matmuls, and PSUM is split 2x[128,2,512] mm-tiles +
  4x[65,512] attention accumulators so consecutive (Q, head-pair) groups
  double-buffer.
"""

import os
import sys

sys.path.insert(0, "/opt/trn_rl_repo")
os.environ.setdefault("MYCRO_LOCAL_CACHE", "1")

import numpy as np
import ml_dtypes

import concourse.bacc as bacc
import concourse.bass as bass
import concourse.mybir as mybir
import concourse.tile as tile
from concourse import bass_utils

# The agent image's antenv lacks axon_hooks, so bass_utils' trace path dies on
# import.  Register a shim module that lazily builds the ctypes NTFF hook.
if "antenv.axon_hooks" not in sys.modules:
    import types

    _shim = types.ModuleType("antenv.axon_hooks")
    _shim._HOOK = None

    def _set_hook(hook, _m=_shim):
        _m._HOOK = hook

    def _get_hook(_m=_shim):
        if _m._HOOK is None:
            try:
                from trn_agent_boot.trn_boot import _ntff_profile_via_ctypes

                _m._HOOK = _ntff_profile_via_ctypes("/opt/axon/libaxon_pjrt.so")
            except Exception:
                _m._HOOK = None
        return _m._HOOK

    _shim.set_axon_ntff_profile_hook = _set_hook
    _shim.get_axon_ntff_profile_hook = _get_hook
    sys.modules["antenv.axon_hooks"] = _shim

B, S, D, H = 2, 2048, 1024, 16
DK = 64                      # head dim
HC = 4                       # heads per core
GC = HC * DK                 # 256 cols per head-group
N_CORES = 8
SCALE = 1.0 / np.sqrt(DK)    # 0.125

F32 = mybir.dt.float32
BF16 = mybir.dt.bfloat16
NP_BF16 = ml_dtypes.bfloat16

TRACE = False
LAST_RESULTS = None


def build_bass():
    nc = bacc.Bacc("TRN2", target_bir_lowering=False, debug=False)

    xT_d = nc.dram_tensor("xT", [D, S], BF16, kind="ExternalInput")
    wqT_d = nc.dram_tensor("wqT", [D, GC], BF16, kind="ExternalInput")
    wkT_d = nc.dram_tensor("wkT", [D, GC], BF16, kind="ExternalInput")
    wvT_d = nc.dram_tensor("wvT", [D, GC], BF16, kind="ExternalInput")
    woT_d = nc.dram_tensor("woT", [GC, D], BF16, kind="ExternalInput")
    mask_d = nc.dram_tensor("mask", [128, 4, 512], BF16, kind="ExternalInput")
    out_d = nc.dram_tensor("out", [S, D], F32, kind="ExternalOutput")

    EXP = mybir.ActivationFunctionType.Exp

    with tile.TileContext(nc) as tc:
        with (
            nc.allow_low_precision(reason="bf16 matmuls, fp32 psum accumulate"),
            tc.tile_pool(name="const", bufs=1) as const,
            tc.tile_pool(name="work", bufs=3) as work,
            tc.tile_pool(name="apool", bufs=2) as apool,
            tc.tile_pool(name="opool", bufs=2) as opool,
            tc.tile_pool(name="rpool", bufs=2) as rpool,
            tc.tile_pool(name="psmm", bufs=2, space="PSUM") as psmm,
            tc.tile_pool(name="psout", bufs=4, space="PSUM") as psout,
        ):
            # ---- load inputs -------------------------------------------------
            xT_dr = xT_d.rearrange("(o p) s -> p o s", p=128)
            xts = []
            for ko in range(8):
                xt = const.tile([128, S], BF16, name=f"xt{ko}")
                nc.sync.dma_start(xt[:], xT_dr[:, ko, :])
                xts.append(xt)
            wq = const.tile([128, 8, GC], BF16)
            nc.gpsimd.dma_start(wq[:], wqT_d.rearrange("(o p) m -> p o m", p=128))
            # descriptor generation for the strided weight loads is slow; put
            # them on the gpsimd queue so they don't serialize behind xT/wq
            wk = const.tile([128, 8, GC], BF16)
            nc.gpsimd.dma_start(wk[:], wkT_d.rearrange("(o p) m -> p o m", p=128))
            wv = const.tile([128, 8, GC], BF16)
            nc.gpsimd.dma_start(wv[:], wvT_d.rearrange("(o p) m -> p o m", p=128))
            wo = const.tile([128, 2, D], BF16)
            nc.gpsimd.dma_start(wo[:], woT_d.rearrange("(o p) n -> p o n", p=128))
            maskt = const.tile([128, 4, 512], BF16)
            nc.gpsimd.dma_start(maskt[:], mask_d[:])

            ones_b = const.tile([128, 64], BF16)
            nc.vector.memset(ones_b[:], 1.0)

            # ---- projections -------------------------------------------------
            # qT/kT: per (head-pair mo, s-half sbh) tiles [128, 1024] so the
            # attention phase can start before all projections finish
            qts = [[const.tile([128, 1024], BF16, name=f"q{m}{s}")
                    for s in range(2)] for m in range(2)]
            kts = [[const.tile([128, 1024], BF16, name=f"k{m}{s}")
                    for s in range(2)] for m in range(2)]
            # v: per j-chunk tiles; per head: 64 value cols + 1 ones col
            vts = []
            for io in range(16):
                vt = const.tile([128, HC * 65], BF16, name=f"v{io}")
                nc.vector.tensor_copy(
                    vt.rearrange("p (h u) -> p h u", u=65)[:, :, 64],
                    ones_b[:, 0:4],
                )
                vts.append(vt)

            for w_sb, dst in ((wq, qts), (wk, kts)):
                for mo in range(2):
                    for sbh in range(2):
                        # one [128,2,512] psum; ko outer so the stationary
                        # weight is reused by the two sb matmuls (1 LDW / 2 MM)
                        ps = psmm.tile([128, 2, 512], F32, tag="mm")
                        for ko in range(8):
                            for sb2 in range(2):
                                sb = 2 * sbh + sb2
                                nc.tensor.matmul(
                                    ps[:, sb2, :],
                                    (w_sb[:, ko, mo * 128:(mo + 1) * 128]),
                                    (xts[ko][:, sb * 512:(sb + 1) * 512]),
                                    start=(ko == 0),
                                    stop=(ko == 7),
                                    skip_group_check=True,
                                )
                        nc.vector.tensor_copy(
                            dst[mo][sbh][:],
                            ps.rearrange("p a n -> p (a n)"),
                        )

            for io in range(16):
                ps = psmm.tile([128, 256], F32, tag="mm")
                for ko in range(8):
                    nc.tensor.matmul(
                        ps[:],
                        (xts[ko][:, io * 128:(io + 1) * 128]),
                        (wv[:, ko, :]),
                        start=(ko == 0),
                        stop=(ko == 7),
                    )
                nc.vector.tensor_copy(
                    vts[io].rearrange("p (h u) -> p h u", u=65)[:, :, 0:64],
                    ps.rearrange("p (h e) -> p h e", e=64),
                )

            # ---- attention + output projection, per 512-query block ---------
            for Q in range(4):
                aT = apool.tile([128, 2, 512], BF16, tag="aT")
                for mo in range(2):
                    nchunks = (Q + 1) * 4
                    out_ps = [
                        psout.tile([65, 512], F32, tag="out", name=f"out_ps{_h}")
                        for _h in range(2)
                    ]
                    for jc in range(nchunks):
                        sc = psmm.tile([128, 2, 512], F32, tag="mm")
                        for hp in range(2):
                            nc.tensor.matmul(
                                sc[:, hp, :],
                                (kts[mo][jc // 8][hp * 64:(hp + 1) * 64,
                                       (jc % 8) * 128:(jc % 8 + 1) * 128]),
                                (qts[mo][Q // 2][hp * 64:(hp + 1) * 64,
                                       (Q % 2) * 512:(Q % 2 + 1) * 512]),
                                start=True,
                                stop=True,
                                skip_group_check=True,
                            )
                        ex = work.tile([128, 2, 512], BF16, tag="exp")
                        nc.scalar.activation(ex[:], sc[:], EXP, scale=SCALE)
                        if jc // 4 == Q:  # diagonal chunk: apply causal mask
                            o = jc - 4 * Q
                            for hp in range(2):
                                nc.vector.tensor_mul(
                                    ex[:, hp, :], ex[:, hp, :], maskt[:, o, :]
                                )
                        for hp in range(2):
                            h = 2 * mo + hp
                            nc.tensor.matmul(
                                out_ps[hp][:],
                                (vts[jc][:, h * 65:(h + 1) * 65]),
                                (ex[:, hp, :]),
                                start=(jc == 0),
                                stop=(jc == nchunks - 1),
                                skip_group_check=True,
                            )
                    for hp in range(2):
                        den = rpool.tile([1, 512], F32, tag="den")
                        nc.vector.tensor_copy(den[:], out_ps[hp][64:65, :])
                        rd_f = rpool.tile([1, 512], F32, tag="rdf")
                        nc.vector.reciprocal_approx_fast(out=rd_f[:], in_=den[:])
                        # broadcast 1/denom across the 64 head-dim partitions
                        # on GpSimd so the PE never sees a K=1 matmul
                        rdb = rpool.tile([64, 512], F32, tag="rdb")
                        nc.gpsimd.partition_broadcast(rdb[:], rd_f[:])
                        nc.vector.tensor_mul(
                            aT[hp * 64:(hp + 1) * 64, mo, :],
                            out_ps[hp][0:64, :],
                            rdb[:],
                        )

                # out-proj for this query block: partial[s, :] = a @ woT
                for so in range(4):
                    osb = opool.tile([128, D], F32, tag="osb")
                    po = psmm.tile([128, 2, 512], F32, tag="mm")
                    for co in range(2):
                        for nt in range(2):
                            nc.tensor.matmul(
                                po[:, nt, :],
                                (aT[:, co, so * 128:(so + 1) * 128]),
                                (wo[:, co, nt * 512:(nt + 1) * 512]),
                                start=(co == 0),
                                stop=(co == 1),
                                skip_group_check=True,
                            )
                    nc.vector.tensor_copy(
                        osb[:], po.rearrange("p a n -> p (a n)")
                    )
                    nc.sync.dma_start(
                        out_d.rearrange("(a p) n -> p a n", p=128)[:, Q * 4 + so, :],
                        osb[:],
                    )

    nc.compile()
    return nc


_NC = None


def _get_nc():
    global _NC
    if _NC is None:
        _NC = build_bass()
    return _NC


def _causal_mask():
    j = np.arange(128)[:, None, None]
    o = np.arange(4)[None, :, None]
    i = np.arange(512)[None, None, :]
    return ((o * 128 + j) <= i).astype(NP_BF16)


def kernel(in_features, Wq, Wk, Wv, Wo):
    global LAST_RESULTS
    nc = _get_nc()

    x = np.asarray(in_features, np.float32)
    Wq = np.asarray(Wq, np.float32)
    Wk = np.asarray(Wk, np.float32)
    Wv = np.asarray(Wv, np.float32)
    Wo = np.asarray(Wo, np.float32)
    mask = _causal_mask()

    in_maps = []
    for c in range(N_CORES):
        b, g = divmod(c, 4)
        cols = slice(g * GC, (g + 1) * GC)
        in_maps.append({
            "xT": np.ascontiguousarray(x[b].T).astype(NP_BF16),
            "wqT": np.ascontiguousarray(Wq.T[:, cols]).astype(NP_BF16),
            "wkT": np.ascontiguousarray(Wk.T[:, cols]).astype(NP_BF16),
            "wvT": np.ascontiguousarray(Wv.T[:, cols]).astype(NP_BF16),
            "woT": np.ascontiguousarray(Wo[:, cols].T).astype(NP_BF16),
            "mask": mask,
        })

    res = bass_utils.run_bass_kernel_spmd(
        nc, in_maps, core_ids=list(range(N_CORES)), trace=TRACE,
    )
    LAST_RESULTS = res
    parts = [res.results[c]["out"] for c in range(N_CORES)]
    out = np.stack([
        parts[4 * b] + parts[4 * b + 1] + parts[4 * b + 2] + parts[4 * b + 3]
        for b in range(B)
    ]).astype(np.float32)
    return out


# revision 3
# speedup vs baseline: 1.1129x; 1.1129x over previous
"""Multi-head self-attention (B=2, S=2048, D=1024, H=16, causal) on 8 TRN2 cores.

Sharding: core c handles batch b=c//4 and head-group g=c%4 (4 heads each).
Host pre-transposes x and the weight slices so the kernel never needs an
on-chip transpose, and pre-converts them to bf16 (PE streams run at
1 cycle/row in bf16; DMA halves):
  xT   [1024, 2048] = x[b].T
  wqT/wkT/wvT [1024, 256] = W.T[:, g*256:(g+1)*256]
  woT  [256, 1024] = Wo[:, g*256:(g+1)*256].T
The kernel writes bf16 partial outputs; host sums the 4 per-group partials
per batch in fp32 at the end.

On-chip dataflow per core:
  qT/kT [256, 2048] (head dim on partitions), v [2048, 4*65] (with a ones
  column appended per head so the PV matmul also accumulates the softmax
  denominator in psum row 64).  Scores are computed transposed
  (scoresT[j, i]) so softmax needs no transpose at all; no max-subtraction
  (scores are O(+-6), exp is safe in fp32).

Pipelining (keeps the PE tensor engine near-continuously busy, which also
holds it at the ramped 2.4 GHz p-state):
  - All PSUM tiles are exactly one 2KB bank: 4-deep "mm" pool (scores /
    projections / out-proj) + 4-deep "out" pool (PV accumulators, so
    consecutive (Q, head-pair) groups double-buffer).
  - Within a query block, PV(jc-1) is emitted AFTER scores(jc), so the PE
    works on the next chunk's scores while the Scalar engine exponentiates
    the current one.
  - The output projection of block Q-1 is interleaved into the first four
    chunks of block Q's attention, removing the Q-boundary PE stall and
    spreading the output DMA.
  - The softmax 1/denominator is broadcast across the 64 head-dim
    partitions on the (otherwise idle) GpSimd engine.
  - Input DMAs are split per-128-row chunk so the first projection matmuls
    start as soon as the first slices land.
"""

import os
import sys

sys.path.insert(0, "/opt/trn_rl_repo")
os.environ.setdefault("MYCRO_LOCAL_CACHE", "1")

import numpy as np
import ml_dtypes

import concourse.bacc as bacc
import concourse.bass as bass
import concourse.mybir as mybir
import concourse.tile as tile
from concourse import bass_utils

# The agent image's antenv lacks axon_hooks, so bass_utils' trace path dies on
# import.  Register a shim module that lazily builds the ctypes NTFF hook.
if "antenv.axon_hooks" not in sys.modules:
    import types

    _shim = types.ModuleType("antenv.axon_hooks")
    _shim._HOOK = None

    def _set_hook(hook, _m=_shim):
        _m._HOOK = hook

    def _get_hook(_m=_shim):
        if _m._HOOK is None:
            try:
                from trn_agent_boot.trn_boot import _ntff_profile_via_ctypes

                _m._HOOK = _ntff_profile_via_ctypes("/opt/axon/libaxon_pjrt.so")
            except Exception:
                _m._HOOK = None
        return _m._HOOK

    _shim.set_axon_ntff_profile_hook = _set_hook
    _shim.get_axon_ntff_profile_hook = _get_hook
    sys.modules["antenv.axon_hooks"] = _shim

B, S, D, H = 2, 2048, 1024, 16
DK = 64                      # head dim
HC = 4                       # heads per core
GC = HC * DK                 # 256 cols per head-group
N_CORES = 8
SCALE = 1.0 / np.sqrt(DK)    # 0.125

F32 = mybir.dt.float32
BF16 = mybir.dt.bfloat16
NP_BF16 = ml_dtypes.bfloat16

TRACE = False
LAST_RESULTS = None


def build_bass():
    nc = bacc.Bacc("TRN2", target_bir_lowering=False, debug=False)

    xT_d = nc.dram_tensor("xT", [D, S], BF16, kind="ExternalInput")
    wqT_d = nc.dram_tensor("wqT", [D, GC], BF16, kind="ExternalInput")
    wkT_d = nc.dram_tensor("wkT", [D, GC], BF16, kind="ExternalInput")
    wvT_d = nc.dram_tensor("wvT", [D, GC], BF16, kind="ExternalInput")
    woT_d = nc.dram_tensor("woT", [GC, D], BF16, kind="ExternalInput")
    mask_d = nc.dram_tensor("mask", [128, 4, 512], BF16, kind="ExternalInput")
    out_d = nc.dram_tensor("out", [S, D], BF16, kind="ExternalOutput")

    EXP = mybir.ActivationFunctionType.Exp

    with tile.TileContext(nc) as tc:
        with (
            nc.allow_low_precision(reason="bf16 matmuls, fp32 psum accumulate"),
            tc.tile_pool(name="const", bufs=1) as const,
            tc.tile_pool(name="work", bufs=4) as work,
            tc.tile_pool(name="apool", bufs=2) as apool,
            tc.tile_pool(name="opool", bufs=2) as opool,
            tc.tile_pool(name="rpool", bufs=2) as rpool,
            tc.tile_pool(name="psmm", bufs=4, space="PSUM") as psmm,
            tc.tile_pool(name="psout", bufs=4, space="PSUM") as psout,
        ):
            # ---- load inputs -------------------------------------------------
            # split per-ko (and per s-half for xT) so the first projection
            # group only gates on a fraction of the input DMA
            xT_dr = xT_d.rearrange("(o p) s -> p o s", p=128)
            xts = []
            for ko in range(8):
                xt = const.tile([128, S], BF16, name=f"xt{ko}")
                nc.sync.dma_start(xt[:, 0:1024], xT_dr[:, ko, 0:1024])
                xts.append(xt)
            for ko in range(8):
                nc.sync.dma_start(xts[ko][:, 1024:2048], xT_dr[:, ko, 1024:2048])
            wq = const.tile([128, 8, GC], BF16)
            wk = const.tile([128, 8, GC], BF16)
            wv = const.tile([128, 8, GC], BF16)
            wqT_dr = wqT_d.rearrange("(o p) m -> p o m", p=128)
            wkT_dr = wkT_d.rearrange("(o p) m -> p o m", p=128)
            wvT_dr = wvT_d.rearrange("(o p) m -> p o m", p=128)
            for ko in range(8):
                nc.gpsimd.dma_start(wq[:, ko, :], wqT_dr[:, ko, :])
                nc.gpsimd.dma_start(wk[:, ko, :], wkT_dr[:, ko, :])
            for ko in range(8):
                nc.gpsimd.dma_start(wv[:, ko, :], wvT_dr[:, ko, :])
            wo = const.tile([128, 2, D], BF16)
            nc.gpsimd.dma_start(wo[:], woT_d.rearrange("(o p) n -> p o n", p=128))
            maskt = const.tile([128, 4, 512], BF16)
            nc.gpsimd.dma_start(maskt[:], mask_d[:])

            ones_b = const.tile([128, 64], BF16)
            nc.vector.memset(ones_b[:], 1.0)

            # ---- projections -------------------------------------------------
            # qT/kT: per (head-pair mo, s-half sbh) tiles [128, 1024] so the
            # attention phase can start before all projections finish
            qts = [[const.tile([128, 1024], BF16, name=f"q{m}{s}")
                    for s in range(2)] for m in range(2)]
            kts = [[const.tile([128, 1024], BF16, name=f"k{m}{s}")
                    for s in range(2)] for m in range(2)]
            # v: per j-chunk tiles; per head: 64 value cols + 1 ones col
            vts = []
            for io in range(16):
                vt = const.tile([128, HC * 65], BF16, name=f"v{io}")
                nc.vector.tensor_copy(
                    vt.rearrange("p (h u) -> p h u", u=65)[:, :, 64],
                    ones_b[:, 0:4],
                )
                vts.append(vt)

            # s-half outer so attention Q=0 (which needs s 0..511 of q and k)
            # unblocks as early as possible
            for sbh in range(2):
                for w_sb, dst in ((wq, qts), (wk, kts)):
                    for mo in range(2):
                        for sb2 in range(2):
                            sb = 2 * sbh + sb2
                            ps = psmm.tile([128, 512], F32, tag="mm")
                            for ko in range(8):
                                nc.tensor.matmul(
                                    ps[:],
                                    (w_sb[:, ko, mo * 128:(mo + 1) * 128]),
                                    (xts[ko][:, sb * 512:(sb + 1) * 512]),
                                    start=(ko == 0),
                                    stop=(ko == 7),
                                    skip_group_check=True,
                                )
                            nc.vector.tensor_copy(
                                dst[mo][sbh][:, sb2 * 512:(sb2 + 1) * 512],
                                ps[:],
                            )

            for io in range(16):
                ps = psmm.tile([128, 256], F32, tag="mm")
                for ko in range(8):
                    nc.tensor.matmul(
                        ps[:],
                        (xts[ko][:, io * 128:(io + 1) * 128]),
                        (wv[:, ko, :]),
                        start=(ko == 0),
                        stop=(ko == 7),
                    )
                nc.vector.tensor_copy(
                    vts[io].rearrange("p (h u) -> p h u", u=65)[:, :, 0:64],
                    ps.rearrange("p (h e) -> p h e", e=64),
                )

            # ---- attention + output projection, per 512-query block ---------
            out_dr = out_d.rearrange("(a p) n -> p a n", p=128)

            def emit_outproj_chunk(Qprev, so, aTprev):
                """partial[s, :] = a @ woT for query sub-block so of Qprev."""
                osb = opool.tile([128, D], BF16, tag="osb", name=f"osb{Qprev}{so}")
                for nt in range(2):
                    po = psmm.tile([128, 512], F32, tag="mm")
                    for co in range(2):
                        nc.tensor.matmul(
                            po[:],
                            (aTprev[:, co, so * 128:(so + 1) * 128]),
                            (wo[:, co, nt * 512:(nt + 1) * 512]),
                            start=(co == 0),
                            stop=(co == 1),
                            skip_group_check=True,
                        )
                    nc.vector.tensor_copy(
                        osb[:, nt * 512:(nt + 1) * 512], po[:]
                    )
                nc.sync.dma_start(out_dr[:, Qprev * 4 + so, :], osb[:])

            aT_prev = None
            for Q in range(4):
                aT = apool.tile([128, 2, 512], BF16, tag="aT")
                for mo in range(2):
                    nchunks = (Q + 1) * 4
                    out_ps = [
                        psout.tile([65, 512], F32, tag="out", name=f"out_ps{_h}")
                        for _h in range(2)
                    ]
                    # software pipeline: PV lags scores by one chunk so the PE
                    # computes scores(jc+1) while Scalar exponentiates chunk jc
                    exs = {}
                    for jc in range(nchunks + 1):
                        if jc < nchunks:
                            for hp in range(2):
                                sc = psmm.tile([128, 512], F32, tag="mm")
                                nc.tensor.matmul(
                                    sc[:],
                                    (kts[mo][jc // 8][hp * 64:(hp + 1) * 64,
                                           (jc % 8) * 128:(jc % 8 + 1) * 128]),
                                    (qts[mo][Q // 2][hp * 64:(hp + 1) * 64,
                                           (Q % 2) * 512:(Q % 2 + 1) * 512]),
                                    start=True,
                                    stop=True,
                                    skip_group_check=True,
                                )
                                ex = work.tile([128, 512], BF16, tag="exp")
                                nc.scalar.activation(ex[:], sc[:], EXP, scale=SCALE)
                                if jc // 4 == Q:  # diagonal: apply causal mask
                                    o = jc - 4 * Q
                                    nc.vector.tensor_mul(
                                        ex[:], ex[:], maskt[:, o, :]
                                    )
                                exs[(jc, hp)] = ex
                            # interleave previous block's output projection
                            # into the first four chunks of mo=0
                            if mo == 0 and jc < 4 and aT_prev is not None:
                                emit_outproj_chunk(Q - 1, jc, aT_prev)
                        jd = jc - 1
                        if jd >= 0:
                            for hp in range(2):
                                h = 2 * mo + hp
                                nc.tensor.matmul(
                                    out_ps[hp][:],
                                    (vts[jd][:, h * 65:(h + 1) * 65]),
                                    (exs.pop((jd, hp))[:]),
                                    start=(jd == 0),
                                    stop=(jd == nchunks - 1),
                                    skip_group_check=True,
                                )
                    for hp in range(2):
                        den = rpool.tile([1, 512], F32, tag="den")
                        nc.vector.tensor_copy(den[:], out_ps[hp][64:65, :])
                        rd_f = rpool.tile([1, 512], F32, tag="rdf")
                        nc.vector.reciprocal_approx_fast(out=rd_f[:], in_=den[:])
                        # broadcast 1/denom across the 64 head-dim partitions
                        # on GpSimd so the PE never sees a K=1 matmul
                        rdb = rpool.tile([64, 512], F32, tag="rdb")
                        nc.gpsimd.partition_broadcast(rdb[:], rd_f[:])
                        nc.vector.tensor_mul(
                            aT[hp * 64:(hp + 1) * 64, mo, :],
                            out_ps[hp][0:64, :],
                            rdb[:],
                        )
                aT_prev = aT

            for so in range(4):  # last block's out-proj (nothing to hide it under)
                emit_outproj_chunk(3, so, aT_prev)

    nc.compile()
    return nc


_NC = None


def _get_nc():
    global _NC
    if _NC is None:
        _NC = build_bass()
    return _NC


def _causal_mask():
    j = np.arange(128)[:, None, None]
    o = np.arange(4)[None, :, None]
    i = np.arange(512)[None, None, :]
    return ((o * 128 + j) <= i).astype(NP_BF16)


def kernel(in_features, Wq, Wk, Wv, Wo):
    global LAST_RESULTS
    nc = _get_nc()

    x = np.asarray(in_features, np.float32)
    Wq = np.asarray(Wq, np.float32)
    Wk = np.asarray(Wk, np.float32)
    Wv = np.asarray(Wv, np.float32)
    Wo = np.asarray(Wo, np.float32)
    mask = _causal_mask()

    in_maps = []
    for c in range(N_CORES):
        b, g = divmod(c, 4)
        cols = slice(g * GC, (g + 1) * GC)
        in_maps.append({
            "xT": np.ascontiguousarray(x[b].T).astype(NP_BF16),
            "wqT": np.ascontiguousarray(Wq.T[:, cols]).astype(NP_BF16),
            "wkT": np.ascontiguousarray(Wk.T[:, cols]).astype(NP_BF16),
            "wvT": np.ascontiguousarray(Wv.T[:, cols]).astype(NP_BF16),
            "woT": np.ascontiguousarray(Wo[:, cols].T).astype(NP_BF16),
            "mask": mask,
        })

    res = bass_utils.run_bass_kernel_spmd(
        nc, in_maps, core_ids=list(range(N_CORES)), trace=TRACE,
    )
    LAST_RESULTS = res
    parts = [res.results[c]["out"].astype(np.float32) for c in range(N_CORES)]
    out = np.stack([
        parts[4 * b] + parts[4 * b + 1] + parts[4 * b + 2] + parts[4 * b + 3]
        for b in range(B)
    ]).astype(np.float32)
    return out


# revision 5
# speedup vs baseline: 1.2046x; 1.0824x over previous
"""Multi-head self-attention (B=2, S=2048, D=1024, H=16, causal) on 8 TRN2 cores.

Sharding: core c handles batch b=c//4 and head-group g=c%4 (4 heads each).
Host pre-transposes x and the weight slices so the kernel never needs an
on-chip transpose, and pre-converts them to bf16 (PE streams run at
1 cycle/row in bf16; DMA halves):
  xT   [1024, 2048] = x[b].T
  wqT/wkT/wvT [1024, 256] = W.T[:, g*256:(g+1)*256]
  woT  [256, 1024] = Wo[:, g*256:(g+1)*256].T
The kernel writes bf16 partial outputs; host sums the 4 per-group partials
per batch in fp32 at the end.

On-chip dataflow per core:
  qT/kT [256, 2048] (head dim on partitions), v [2048, 4*65] (with a ones
  column appended per head so the PV matmul also accumulates the softmax
  denominator in psum row 64).  Scores are computed transposed
  (scoresT[j, i]) so softmax needs no transpose at all; no max-subtraction
  (scores are O(+-6), exp is safe in fp32).

Performance notes (the PE tensor engine only reaches its ramped 2.4 GHz
p-state after ~3us of *continuous* work; any idle resets it to 1.2 GHz, so
everything is built around never letting the PE wait):
  - Attention runs a lag-2 software pipeline: scores(jc) are emitted two
    chunks ahead of PV(jc), so the PE has ~1.3us of score matmuls in its
    queue while the Scalar engine exponentiates a chunk.
  - Causal trimming: for the 4 diagonal j-chunks of each query block the
    scores / exp / PV are restricted to the query range [o*128, 512) that
    can actually attend to that chunk; the causal mask reduces to a single
    [128,128] lower-triangular multiply per diagonal chunk.
  - All PSUM tiles are one 2KB bank: a 6-deep "mm" pool (scores, QKV
    projections, out-proj) + 2 PV accumulators.
  - The output projection of block Q-1 is interleaved two sub-blocks at a
    time into the starts of block Q's two head-pair groups, filling the
    PE while the softmax denominators of the previous group are applied.
  - The softmax 1/denominator is broadcast across the 64 head-dim
    partitions on the (otherwise idle) GpSimd engine.
  - xT is DMAed in four column-quarters and the first projection group
    only needs the first quarter; DMA descriptor generation (~0.6us per
    transfer, serial per issuing queue) is kept off the critical path.
"""

import os
import sys

sys.path.insert(0, "/opt/trn_rl_repo")
os.environ.setdefault("MYCRO_LOCAL_CACHE", "1")

import numpy as np
import ml_dtypes

import concourse.bacc as bacc
import concourse.bass as bass
import concourse.mybir as mybir
import concourse.tile as tile
from concourse import bass_utils

# The agent image's antenv lacks axon_hooks, so bass_utils' trace path dies on
# import.  Register a shim module that lazily builds the ctypes NTFF hook.
if "antenv.axon_hooks" not in sys.modules:
    import types

    _shim = types.ModuleType("antenv.axon_hooks")
    _shim._HOOK = None

    def _set_hook(hook, _m=_shim):
        _m._HOOK = hook

    def _get_hook(_m=_shim):
        if _m._HOOK is None:
            try:
                from trn_agent_boot.trn_boot import _ntff_profile_via_ctypes

                _m._HOOK = _ntff_profile_via_ctypes("/opt/axon/libaxon_pjrt.so")
            except Exception:
                _m._HOOK = None
        return _m._HOOK

    _shim.set_axon_ntff_profile_hook = _set_hook
    _shim.get_axon_ntff_profile_hook = _get_hook
    sys.modules["antenv.axon_hooks"] = _shim

B, S, D, H = 2, 2048, 1024, 16
DK = 64                      # head dim
HC = 4                       # heads per core
GC = HC * DK                 # 256 cols per head-group
N_CORES = 8
SCALE = 1.0 / np.sqrt(DK)    # 0.125

F32 = mybir.dt.float32
BF16 = mybir.dt.bfloat16
NP_BF16 = ml_dtypes.bfloat16

TRACE = False
LAST_RESULTS = None


def build_bass():
    nc = bacc.Bacc("TRN2", target_bir_lowering=False, debug=False)

    xT_d = nc.dram_tensor("xT", [D, S], BF16, kind="ExternalInput")
    wqT_d = nc.dram_tensor("wqT", [D, GC], BF16, kind="ExternalInput")
    wkT_d = nc.dram_tensor("wkT", [D, GC], BF16, kind="ExternalInput")
    wvT_d = nc.dram_tensor("wvT", [D, GC], BF16, kind="ExternalInput")
    woT_d = nc.dram_tensor("woT", [GC, D], BF16, kind="ExternalInput")
    mask_d = nc.dram_tensor("mask", [128, 128], BF16, kind="ExternalInput")
    out_d = nc.dram_tensor("out", [S, D], BF16, kind="ExternalOutput")

    EXP = mybir.ActivationFunctionType.Exp

    with tile.TileContext(nc) as tc:
        with (
            nc.allow_low_precision(reason="bf16 matmuls, fp32 psum accumulate"),
            tc.tile_pool(name="const", bufs=1) as const,
            tc.tile_pool(name="work", bufs=6) as work,
            tc.tile_pool(name="apool", bufs=2) as apool,
            tc.tile_pool(name="opool", bufs=2) as opool,
            tc.tile_pool(name="rpool", bufs=2) as rpool,
            tc.tile_pool(name="pmm", bufs=6, space="PSUM") as pmm,
            tc.tile_pool(name="psout", bufs=2, space="PSUM") as psout,
        ):
            # ---- load inputs -------------------------------------------------
            xT_dr = xT_d.rearrange("(o p) s -> p o s", p=128)
            xts = const.tile([128, 8, S], BF16, name="xts")
            for quarter in range(4):
                s0 = quarter * 512
                nc.sync.dma_start(
                    xts[:, :, s0:s0 + 512], xT_dr[:, :, s0:s0 + 512]
                )
            wq = const.tile([128, 8, GC], BF16)
            nc.gpsimd.dma_start(wq[:], wqT_d.rearrange("(o p) m -> p o m", p=128))
            wk = const.tile([128, 8, GC], BF16)
            nc.gpsimd.dma_start(wk[:], wkT_d.rearrange("(o p) m -> p o m", p=128))
            wv = const.tile([128, 8, GC], BF16)
            nc.gpsimd.dma_start(wv[:], wvT_d.rearrange("(o p) m -> p o m", p=128))
            wo = const.tile([128, 2, D], BF16)
            nc.gpsimd.dma_start(wo[:], woT_d.rearrange("(o p) n -> p o n", p=128))
            maskt = const.tile([128, 128], BF16)
            nc.gpsimd.dma_start(maskt[:], mask_d[:])

            ones_b = const.tile([128, 64], BF16)
            nc.vector.memset(ones_b[:], 1.0)

            # ---- projections -------------------------------------------------
            # qT/kT: per (head-pair mo, s-half sbh) tiles [128, 1024] so the
            # attention phase can start before all projections finish
            qts = [[const.tile([128, 1024], BF16, name=f"q{m}{s}")
                    for s in range(2)] for m in range(2)]
            kts = [[const.tile([128, 1024], BF16, name=f"k{m}{s}")
                    for s in range(2)] for m in range(2)]
            # v: per j-chunk tiles; per head: 64 value cols + 1 ones col
            vts = []
            for io in range(16):
                vt = const.tile([128, HC * 65], BF16, name=f"v{io}")
                nc.vector.tensor_copy(
                    vt.rearrange("p (h u) -> p h u", u=65)[:, :, 64],
                    ones_b[:, 0:4],
                )
                vts.append(vt)

            # s-quarter outer: each quarter's groups only gate on that
            # quarter's slice of the xT DMA
            for sb in range(4):
                for w_sb, dst in ((wq, qts), (wk, kts)):
                    for mo in range(2):
                        ps = pmm.tile([128, 512], F32, tag="mm")
                        for ko in range(8):
                            nc.tensor.matmul(
                                ps[:],
                                (w_sb[:, ko, mo * 128:(mo + 1) * 128]),
                                (xts[:, ko, sb * 512:(sb + 1) * 512]),
                                start=(ko == 0),
                                stop=(ko == 7),
                                skip_group_check=True,
                            )
                        nc.vector.tensor_copy(
                            dst[mo][sb // 2][:, (sb % 2) * 512:(sb % 2 + 1) * 512],
                            ps[:],
                        )

            for io in range(16):
                ps = pmm.tile([128, 256], F32, tag="mm")
                for ko in range(8):
                    nc.tensor.matmul(
                        ps[:],
                        (xts[:, ko, io * 128:(io + 1) * 128]),
                        (wv[:, ko, :]),
                        start=(ko == 0),
                        stop=(ko == 7),
                    )
                nc.vector.tensor_copy(
                    vts[io].rearrange("p (h u) -> p h u", u=65)[:, :, 0:64],
                    ps.rearrange("p (h e) -> p h e", e=64),
                )

            # ---- attention + output projection, per 512-query block ---------
            out_dr = out_d.rearrange("(a p) n -> p a n", p=128)

            def emit_outproj_chunk(Qprev, so, aTprev):
                """partial[s, :] = a @ woT for query sub-block so of Qprev."""
                osb = opool.tile([128, D], BF16, tag="osb", name=f"osb{Qprev}{so}")
                for nt in range(2):
                    po = pmm.tile([128, 512], F32, tag="mm")
                    for co in range(2):
                        nc.tensor.matmul(
                            po[:],
                            (aTprev[:, co, so * 128:(so + 1) * 128]),
                            (wo[:, co, nt * 512:(nt + 1) * 512]),
                            start=(co == 0),
                            stop=(co == 1),
                            skip_group_check=True,
                        )
                    nc.vector.tensor_copy(
                        osb[:, nt * 512:(nt + 1) * 512], po[:]
                    )
                nc.sync.dma_start(out_dr[:, Qprev * 4 + so, :], osb[:])

            LAG = 2
            aT_prev = None
            for Q in range(4):
                aT = apool.tile([128, 2, 512], BF16, tag="aT")
                for mo in range(2):
                    nchunks = (Q + 1) * 4
                    out_ps = [
                        psout.tile([65, 512], F32, tag="out", name=f"out_ps{_h}")
                        for _h in range(2)
                    ]
                    exs = {}

                    def chunk_lo(jc):
                        # diagonal chunks only see queries >= (jc-4Q)*128
                        return (jc - 4 * Q) * 128 if jc >= 4 * Q else 0

                    for jc in range(nchunks + LAG):
                        if jc < nchunks:
                            lo = chunk_lo(jc)
                            for hp in range(2):
                                sc = pmm.tile([128, 512], F32, tag="mm")
                                nc.tensor.matmul(
                                    sc[:, lo:512],
                                    (kts[mo][jc // 8][hp * 64:(hp + 1) * 64,
                                           (jc % 8) * 128:(jc % 8 + 1) * 128]),
                                    (qts[mo][Q // 2][hp * 64:(hp + 1) * 64,
                                           (Q % 2) * 512 + lo:(Q % 2 + 1) * 512]),
                                    start=True,
                                    stop=True,
                                    skip_group_check=True,
                                )
                                ex = work.tile([128, 512], BF16, tag="exp")
                                nc.scalar.activation(
                                    ex[:, lo:512], sc[:, lo:512], EXP, scale=SCALE
                                )
                                if lo > 0 or jc == 4 * Q:
                                    # triangular mask on the 128-col diag band
                                    nc.vector.tensor_mul(
                                        ex[:, lo:lo + 128],
                                        ex[:, lo:lo + 128],
                                        maskt[:],
                                    )
                                exs[(jc, hp)] = ex
                            # interleave previous block's output projection:
                            # two sub-blocks at the start of each mo group
                            if jc < 2 and aT_prev is not None:
                                emit_outproj_chunk(Q - 1, 2 * mo + jc, aT_prev)
                        jd = jc - LAG
                        if jd >= 0:
                            lo = chunk_lo(jd)
                            for hp in range(2):
                                h = 2 * mo + hp
                                nc.tensor.matmul(
                                    out_ps[hp][:, lo:512],
                                    (vts[jd][:, h * 65:(h + 1) * 65]),
                                    (exs.pop((jd, hp))[:, lo:512]),
                                    start=(jd == 0),
                                    stop=(jd == nchunks - 1),
                                    skip_group_check=True,
                                )
                    for hp in range(2):
                        den = rpool.tile([1, 512], F32, tag="den")
                        nc.vector.tensor_copy(den[:], out_ps[hp][64:65, :])
                        rd_f = rpool.tile([1, 512], F32, tag="rdf")
                        nc.vector.reciprocal_approx_fast(out=rd_f[:], in_=den[:])
                        # broadcast 1/denom across the 64 head-dim partitions
                        # on GpSimd so the PE never sees a K=1 matmul
                        rdb = rpool.tile([64, 512], F32, tag="rdb")
                        nc.gpsimd.partition_broadcast(rdb[:], rd_f[:])
                        nc.vector.tensor_mul(
                            aT[hp * 64:(hp + 1) * 64, mo, :],
                            out_ps[hp][0:64, :],
                            rdb[:],
                        )
                aT_prev = aT

            for so in range(4):  # last block's out-proj (nothing to hide it under)
                emit_outproj_chunk(3, so, aT_prev)

    nc.compile()
    return nc


_NC = None


def _get_nc():
    global _NC
    if _NC is None:
        _NC = build_bass()
    return _NC


def _causal_mask():
    j = np.arange(128)[:, None]
    i = np.arange(128)[None, :]
    return (j <= i).astype(NP_BF16)


def kernel(in_features, Wq, Wk, Wv, Wo):
    global LAST_RESULTS
    nc = _get_nc()

    x = np.asarray(in_features, np.float32)
    Wq = np.asarray(Wq, np.float32)
    Wk = np.asarray(Wk, np.float32)
    Wv = np.asarray(Wv, np.float32)
    Wo = np.asarray(Wo, np.float32)
    mask = _causal_mask()

    in_maps = []
    for c in range(N_CORES):
        b, g = divmod(c, 4)
        cols = slice(g * GC, (g + 1) * GC)
        in_maps.append({
            "xT": np.ascontiguousarray(x[b].T).astype(NP_BF16),
            "wqT": np.ascontiguousarray(Wq.T[:, cols]).astype(NP_BF16),
            "wkT": np.ascontiguousarray(Wk.T[:, cols]).astype(NP_BF16),
            "wvT": np.ascontiguousarray(Wv.T[:, cols]).astype(NP_BF16),
            "woT": np.ascontiguousarray(Wo[:, cols].T).astype(NP_BF16),
            "mask": mask,
        })

    res = bass_utils.run_bass_kernel_spmd(
        nc, in_maps, core_ids=list(range(N_CORES)), trace=TRACE,
    )
    LAST_RESULTS = res
    parts = [res.results[c]["out"].astype(np.float32) for c in range(N_CORES)]
    out = np.stack([
        parts[4 * b] + parts[4 * b + 1] + parts[4 * b + 2] + parts[4 * b + 3]
        for b in range(B)
    ]).astype(np.float32)
    return out
